# revision 10
# baseline (speedup 1.0000x reference)
"""Trainium2 Bass kernel for a pre-norm decoder block with GQA attention + top-2 MoE.

Strategy (8 NeuronCores):
  Launch A (attention): shard by (batch, kv-group): core c -> batch c//4, group c%4.
    Each core computes RMSNorm+RoPE for its batch, Q/K/V for its 4 heads /
    1 kv-group over all 2048 tokens, causal attention, and the partial
    output projection attn_g @ Wo[group rows].  Partials are summed with a
    ReduceScatter over the 4 cores of each batch, so core c ends with the
    attention-block output h for its quarter of the sequence, plus the
    RMSNorm'd y (transposed) and router logits for those tokens.
  Host: top-2 routing on the logits (pure index/gather glue), builds the
    per-expert token batches (expert parallelism, "all-to-all" dispatch done
    as the launch-B input sharding).
  Launch B (MoE FFN): core e -> expert e.  Dense SwiGLU FFN over the tokens
    routed to that expert (padded to capacity C), gate-scaled on device.
  Host: scatter-add the gated expert outputs back and add the residual.

Matmuls run with float32 storage; TensorEngine dtype is configurable below
(float32r = full-rate fp32 mode).
"""

import math
import os
from contextlib import ExitStack

import numpy as np

import concourse.bass as bass
import concourse.mybir as mybir
import concourse.tile as tile
from concourse import bacc
from concourse.bass_utils import run_bass_kernel_spmd
from concourse.kernels.tile_matmul import matmul_tile_kernel
from concourse.masks import make_causal_mask, make_identity

AF = mybir.ActivationFunctionType
F32 = mybir.dt.float32
X = mybir.AxisListType.X

# model dims (hardcoded for this problem)
B, S, D = 2, 2048, 1024
NH, G, DK = 16, 4, 64
HPG = NH // G          # 4 heads per group
GD = HPG * DK          # 256 = per-group head width
FF, E = 4096, 8
EPS = 1e-6
N_CORES = 8
SQ = S // 4            # 512 tokens per core after reduce-scatter
NT = S // 128          # 16 token tiles per batch

# TensorEngine compute dtype for the heavy matmuls (storage stays fp32).
MM_DT = mybir.dt.float32r
# dtype for attention score/AV matmuls
ATT_DT = mybir.dt.float32r

_CACHE = {}


def _mm(ap, dt=MM_DT):
    return ap.bitcast(dt) if dt != F32 else ap


# ---------------------------------------------------------------- launch A

def _rmsnorm_tile(nc, pool, src_ap, wbc, tag, eps_ap):
    """token-major rmsnorm of a [128, D] tile; returns normalized sbuf tile."""
    sq = pool.tile([128, D], F32, tag=f"{tag}_sq")
    nc.vector.tensor_mul(sq[:], src_ap, src_ap)
    ssum = pool.tile([128, 1], F32, tag=f"{tag}_ssum")
    nc.vector.reduce_sum(ssum[:], sq[:], axis=X)
    rms = pool.tile([128, 1], F32, tag=f"{tag}_rms")
    nc.scalar.activation(rms[:], ssum[:], AF.Sqrt, bias=eps_ap, scale=1.0 / D)
    rinv = pool.tile([128, 1], F32, tag=f"{tag}_rinv")
    nc.vector.reciprocal(rinv[:], rms[:])
    xn = pool.tile([128, D], F32, tag=f"{tag}_xn")
    nc.scalar.activation(xn[:], src_ap, AF.Copy, scale=rinv[:])
    nc.vector.tensor_mul(xn[:], xn[:], wbc[:])
    return xn


def build_attn():
    nc = bacc.Bacc("TRN2", target_bir_lowering=False, debug=False,
                   num_devices=N_CORES)
    x_b = nc.declare_dram_parameter("x_b", [S, D], F32, False)
    x_mine = nc.declare_dram_parameter("x_mine", [SQ, D], F32, False)
    Wq_g = nc.declare_dram_parameter("Wq_g", [D, GD], F32, False)
    Wk_g = nc.declare_dram_parameter("Wk_g", [D, DK], F32, False)
    Wv_g = nc.declare_dram_parameter("Wv_g", [D, DK], F32, False)
    Wo_g = nc.declare_dram_parameter("Wo_g", [GD, D], F32, False)
    n1w = nc.declare_dram_parameter("n1w", [1, D], F32, False)
    n2w = nc.declare_dram_parameter("n2w", [1, D], F32, False)
    cos_t = nc.declare_dram_parameter("cos_t", [S, D // 2], F32, False)
    sin_t = nc.declare_dram_parameter("sin_t", [S, D // 2], F32, False)
    rw = nc.declare_dram_parameter("rw", [D, E], F32, False)
    rb = nc.declare_dram_parameter("rb", [E, 1], F32, False)
    h_out = nc.declare_dram_parameter("h_out", [SQ, D], F32, True)
    yt_out = nc.declare_dram_parameter("yt_out", [D, SQ], F32, True)
    lg_out = nc.declare_dram_parameter("lg_out", [E, SQ], F32, True)

    with tile.TileContext(nc) as tc:
        with ExitStack() as top:
            dram = top.enter_context(tc.tile_pool(name="dram", bufs=1, space="DRAM"))
            xr_t = dram.tile([D, S], F32)
            q_t = dram.tile([GD, S], F32)
            k_t = dram.tile([DK, S], F32)
            v_tm = dram.tile([S, DK], F32)
            attn_t = dram.tile([GD, S], F32)
            partial_o = dram.tile([S, D], F32)
            rs_out = dram.tile([SQ, D], F32)

            cpool = top.enter_context(tc.tile_pool(name="const", bufs=1))
            ident = cpool.tile([128, 128], F32)
            make_identity(nc, ident[:])
            dmask = cpool.tile([128, 128], F32)
            make_causal_mask(nc, dmask[:], mask_val=-1e9)
            w1r = cpool.tile([1, D], F32)
            nc.sync.dma_start(w1r[:], n1w.ap())
            w1bc = cpool.tile([128, D], F32)
            nc.gpsimd.partition_broadcast(w1bc[:], w1r[:1, :])
            w2r = cpool.tile([1, D], F32)
            nc.sync.dma_start(w2r[:], n2w.ap())
            w2bc = cpool.tile([128, D], F32)
            nc.gpsimd.partition_broadcast(w2bc[:], w2r[:1, :])
            rw_sb = cpool.tile([128, 8 * E], ATT_DT)
            for kd in range(8):
                nc.sync.dma_start(rw_sb[:, kd * E:(kd + 1) * E],
                                  _mm(rw.ap()[kd * 128:(kd + 1) * 128, :], ATT_DT))
            rb_sb = cpool.tile([E, 1], F32)
            nc.sync.dma_start(rb_sb[:], rb.ap())
            eps_sb = cpool.tile([128, 1], F32)
            nc.gpsimd.memset(eps_sb[:], float(EPS))

            # ---- phase 1: rmsnorm + rope + transpose -> xr_t [D, S]
            with ExitStack() as ctx:
                pool = ctx.enter_context(tc.tile_pool(name="nrm", bufs=3))
                pp = ctx.enter_context(tc.tile_pool(name="nrm_ps", bufs=4, space="PSUM"))
                for it in range(NT):
                    xt = pool.tile([128, D], F32, tag="xt")
                    nc.sync.dma_start(xt[:], x_b.ap()[it * 128:(it + 1) * 128, :])
                    xn = _rmsnorm_tile(nc, pool, xt[:], w1bc, "n1", eps_sb[:])
                    cos = pool.tile([128, D // 2], F32, tag="cos")
                    nc.sync.dma_start(cos[:], cos_t.ap()[it * 128:(it + 1) * 128, :])
                    sin = pool.tile([128, D // 2], F32, tag="sin")
                    nc.sync.dma_start(sin[:], sin_t.ap()[it * 128:(it + 1) * 128, :])
                    xe, xo = xn[:, 0::2], xn[:, 1::2]
                    xr = pool.tile([128, D], F32, tag="xr")
                    t1 = pool.tile([128, D // 2], F32, tag="t1")
                    t2 = pool.tile([128, D // 2], F32, tag="t2")
                    nc.vector.tensor_mul(t1[:], xe, cos[:])
                    nc.vector.tensor_mul(t2[:], xo, sin[:])
                    nc.vector.tensor_sub(xr[:, :D // 2], t1[:], t2[:])
                    nc.vector.tensor_mul(t1[:], xe, sin[:])
                    nc.vector.tensor_mul(t2[:], xo, cos[:])
                    nc.vector.tensor_add(xr[:, D // 2:], t1[:], t2[:])
                    for kd in range(8):
                        pt = pp.tile([128, 128], F32, tag="pt")
                        nc.tensor.transpose(pt[:], xr[:, kd * 128:(kd + 1) * 128], ident[:])
                        st = pool.tile([128, 128], F32, tag="st")
                        nc.scalar.copy(st[:], pt[:])
                        nc.sync.dma_start(
                            xr_t[kd * 128:(kd + 1) * 128, it * 128:(it + 1) * 128], st[:])

            # ---- phase 2: projections
            matmul_tile_kernel(
                tc, _mm(Wq_g.ap()), _mm(xr_t[:]), q_t[:],
                post_mxn_tile_fn=lambda nc_, sb, md, _:
                    nc_.scalar.mul(sb, sb, 1.0 / math.sqrt(DK)))
            matmul_tile_kernel(tc, _mm(Wk_g.ap()), _mm(xr_t[:]), k_t[:])
            matmul_tile_kernel(tc, _mm(xr_t[:]), _mm(Wv_g.ap()), v_tm[:])

            # ---- phase 3: causal attention for 4 heads
            with ExitStack() as ctx:
                apool = ctx.enter_context(tc.tile_pool(name="att_in", bufs=1))
                spool = ctx.enter_context(tc.tile_pool(name="att_wk", bufs=3))
                pp_s = ctx.enter_context(tc.tile_pool(name="ps_s", bufs=2, space="PSUM"))
                pp_t = ctx.enter_context(tc.tile_pool(name="ps_t", bufs=2, space="PSUM"))
                pp_av = ctx.enter_context(tc.tile_pool(name="ps_av", bufs=2, space="PSUM"))

                q_sb = []
                for h in range(HPG):
                    qh = apool.tile([DK, S], ATT_DT, name=f"q_sb{h}")
                    nc.sync.dma_start(qh[:], _mm(q_t[h * DK:(h + 1) * DK, :], ATT_DT))
                    q_sb.append(qh)
                k_sb = apool.tile([DK, S], ATT_DT)
                nc.sync.dma_start(k_sb[:], _mm(k_t[:], ATT_DT))
                v_sb = apool.tile([128, NT * DK], ATT_DT)
                for kt in range(NT):
                    nc.sync.dma_start(v_sb[:, kt * DK:(kt + 1) * DK],
                                      _mm(v_tm[kt * 128:(kt + 1) * 128, :], ATT_DT))

                for j in range(NT):
                    S_k = (j + 1) * 128
                    attn_sb = spool.tile([128, 2 * 128], F32, tag="attn_sb")
                    for h in range(HPG):
                        sc = spool.tile([128, S], F32, tag="sc")
                        for c0 in range(0, S_k, 512):
                            w = min(512, S_k - c0)
                            ps = pp_s.tile([128, 512], F32, tag="ps")
                            nc.tensor.matmul(
                                ps[:, :w],
                                q_sb[h][:, j * 128:(j + 1) * 128],
                                k_sb[:, c0:c0 + w],
                                start=True, stop=True)
                            if c0 + w == S_k:
                                if w > 128:
                                    nc.scalar.copy(sc[:, c0:c0 + w - 128], ps[:, :w - 128])
                                nc.vector.tensor_add(sc[:, S_k - 128:S_k],
                                                     ps[:, w - 128:w], dmask[:])
                            else:
                                nc.scalar.copy(sc[:, c0:c0 + w], ps[:, :w])
                        negm = spool.tile([128, 1], F32, tag="negm")
                        nc.vector.reduce_max(negm[:], sc[:, :S_k], axis=X, negate=True)
                        nc.scalar.activation(sc[:, :S_k], sc[:, :S_k], AF.Exp, bias=negm[:])
                        ssum = spool.tile([128, 1], F32, tag="ssum2")
                        nc.vector.reduce_sum(ssum[:], sc[:, :S_k], axis=X)
                        rinv = spool.tile([128, 1], F32, tag="rinv2")
                        nc.vector.reciprocal(rinv[:], ssum[:])
                        nc.scalar.activation(sc[:, :S_k], sc[:, :S_k], AF.Copy, scale=rinv[:])
                        ptb = spool.tile([128, S], ATT_DT, tag="ptb")
                        for kt in range(j + 1):
                            pst = pp_t.tile([128, 128], F32, tag="pst")
                            nc.tensor.transpose(pst[:], sc[:, kt * 128:(kt + 1) * 128],
                                                ident[:])
                            nc.scalar.copy(ptb[:, kt * 128:(kt + 1) * 128], pst[:])
                        av = pp_av.tile([DK, 128], F32, tag="av")
                        for kt in range(j + 1):
                            nc.tensor.matmul(
                                av[:],
                                v_sb[:, kt * DK:(kt + 1) * DK],
                                ptb[:, kt * 128:(kt + 1) * 128],
                                start=(kt == 0), stop=(kt == j))
                        nc.scalar.copy(
                            attn_sb[(h % 2) * DK:(h % 2 + 1) * DK,
                                    (h // 2) * 128:(h // 2 + 1) * 128], av[:])
                    nc.sync.dma_start(attn_t[0:128, j * 128:(j + 1) * 128],
                                      attn_sb[:, 0:128])
                    nc.sync.dma_start(attn_t[128:256, j * 128:(j + 1) * 128],
                                      attn_sb[:, 128:256])

            # ---- phase 4: partial output projection
            matmul_tile_kernel(tc, _mm(attn_t[:]), _mm(Wo_g.ap()), partial_o[:])

            # ---- phase 5: reduce-scatter over the 4 cores of each batch
            nc.gpsimd.collective_compute(
                "ReduceScatter", mybir.AluOpType.add,
                replica_groups=[[0, 1, 2, 3], [4, 5, 6, 7]],
                ins=[partial_o[:].opt()], outs=[rs_out[:].opt()])

            # ---- phase 6: h = x + attn; y = rmsnorm(h); y_t; logits
            with ExitStack() as ctx:
                pool = ctx.enter_context(tc.tile_pool(name="tail", bufs=3))
                pp = ctx.enter_context(tc.tile_pool(name="tail_ps", bufs=4, space="PSUM"))
                pp_l = ctx.enter_context(tc.tile_pool(name="tail_pl", bufs=2, space="PSUM"))
                for it in range(SQ // 128):
                    ht = pool.tile([128, D], F32, tag="ht")
                    nc.sync.dma_start(ht[:], rs_out[it * 128:(it + 1) * 128, :])
                    xm = pool.tile([128, D], F32, tag="xm")
                    nc.sync.dma_start(xm[:], x_mine.ap()[it * 128:(it + 1) * 128, :])
                    nc.vector.tensor_add(ht[:], ht[:], xm[:])
                    nc.sync.dma_start(h_out.ap()[it * 128:(it + 1) * 128, :], ht[:])
                    yt = _rmsnorm_tile(nc, pool, ht[:], w2bc, "n2", eps_sb[:])
                    st = pool.tile([128, 8 * 128], ATT_DT, tag="st2")
                    for kd in range(8):
                        pt = pp.tile([128, 128], F32, tag="pt2")
                        nc.tensor.transpose(pt[:], yt[:, kd * 128:(kd + 1) * 128], ident[:])
                        nc.scalar.copy(st[:, kd * 128:(kd + 1) * 128], pt[:])
                        nc.sync.dma_start(
                            _mm(yt_out.ap()[kd * 128:(kd + 1) * 128,
                                            it * 128:(it + 1) * 128], ATT_DT),
                            st[:, kd * 128:(kd + 1) * 128])
                    psl = pp_l.tile([E, 128], F32, tag="pl")
                    for kd in range(8):
                        nc.tensor.matmul(
                            psl[:],
                            rw_sb[:, kd * E:(kd + 1) * E],
                            st[:, kd * 128:(kd + 1) * 128],
                            start=(kd == 0), stop=(kd == 7))
                    lg = pool.tile([E, 128], F32, tag="lg")
                    nc.vector.tensor_scalar_add(lg[:], psl[:], rb_sb[:, 0:1])
                    nc.sync.dma_start(lg_out.ap()[:, it * 128:(it + 1) * 128], lg[:])

    nc.compile()
    return nc


# ---------------------------------------------------------------- launch B

def build_moe(C):
    nc = bacc.Bacc("TRN2", target_bir_lowering=False, debug=False,
                   num_devices=N_CORES)
    Yt = nc.declare_dram_parameter("Yt", [D, C], F32, False)
    gates = nc.declare_dram_parameter("gates", [1, C], F32, False)
    W1 = nc.declare_dram_parameter("W1", [D, FF], F32, False)
    W2 = nc.declare_dram_parameter("W2", [D, FF], F32, False)
    W3 = nc.declare_dram_parameter("W3", [FF, D], F32, False)
    b1 = nc.declare_dram_parameter("b1", [128, FF // 128], F32, False)
    b2 = nc.declare_dram_parameter("b2", [128, FF // 128], F32, False)
    b3 = nc.declare_dram_parameter("b3", [128, D // 128], F32, False)
    ot_out = nc.declare_dram_parameter("ot_out", [D, C], F32, True)

    KD = D // 128     # 8 k-tiles over D
    MF = FF // 128    # 32 m-tiles over FF
    nch = [(c0, min(512, C - c0)) for c0 in range(0, C, 512)]

    with tile.TileContext(nc) as tc:
        with ExitStack() as top:
            dram = top.enter_context(tc.tile_pool(name="dram", bufs=1, space="DRAM"))
            ht = dram.tile([FF, C], F32)

            cpool = top.enter_context(tc.tile_pool(name="const", bufs=1))
            b1_sb = cpool.tile([128, MF], F32)
            nc.sync.dma_start(b1_sb[:], b1.ap())
            b2_sb = cpool.tile([128, MF], F32)
            nc.sync.dma_start(b2_sb[:], b2.ap())
            b3_sb = cpool.tile([128, D // 128], F32)
            nc.sync.dma_start(b3_sb[:], b3.ap())
            g_row = cpool.tile([1, C], F32)
            nc.sync.dma_start(g_row[:], gates.ap())
            g_bc = cpool.tile([128, C], F32)
            nc.gpsimd.partition_broadcast(g_bc[:], g_row[:1, :])

            # ---- pass 1: h = silu(W1^T y + b1) * (W2^T y + b2) -> ht [FF, C]
            with ExitStack() as ctx:
                ypool = ctx.enter_context(tc.tile_pool(name="yp", bufs=1))
                yt_sb = ypool.tile([128, KD * C], MM_DT)
                for k in range(KD):
                    nc.sync.dma_start(yt_sb[:, k * C:(k + 1) * C],
                                      _mm(Yt.ap()[k * 128:(k + 1) * 128, :]))
                wpool = ctx.enter_context(tc.tile_pool(name="wp", bufs=3))
                hpool = ctx.enter_context(tc.tile_pool(name="hp", bufs=3))
                pps = ctx.enter_context(tc.tile_pool(name="pps", bufs=4, space="PSUM"))
                for m in range(MF):
                    w1t = wpool.tile([128, KD * 128], MM_DT, tag="w1t")
                    w2t = wpool.tile([128, KD * 128], MM_DT, tag="w2t")
                    for k in range(KD):
                        nc.sync.dma_start(
                            w1t[:, k * 128:(k + 1) * 128],
                            _mm(W1.ap()[k * 128:(k + 1) * 128, m * 128:(m + 1) * 128]))
                        nc.sync.dma_start(
                            w2t[:, k * 128:(k + 1) * 128],
                            _mm(W2.ap()[k * 128:(k + 1) * 128, m * 128:(m + 1) * 128]))
                    for c0, w in nch:
                        ps1 = pps.tile([128, 512], F32, tag="ps1")
                        for k in range(KD):
                            nc.tensor.matmul(
                                ps1[:, :w],
                                w1t[:, k * 128:(k + 1) * 128],
                                yt_sb[:, k * C + c0:k * C + c0 + w],
                                start=(k == 0), stop=(k == KD - 1))
                        ps2 = pps.tile([128, 512], F32, tag="ps2")
                        for k in range(KD):
                            nc.tensor.matmul(
                                ps2[:, :w],
                                w2t[:, k * 128:(k + 1) * 128],
                                yt_sb[:, k * C + c0:k * C + c0 + w],
                                start=(k == 0), stop=(k == KD - 1))
                        s_sb = hpool.tile([128, 512], F32, tag="s_sb")
                        nc.scalar.activation(s_sb[:, :w], ps1[:, :w], AF.Silu,
                                             bias=b1_sb[:, m:m + 1])
                        t2 = hpool.tile([128, 512], F32, tag="t2")
                        nc.vector.tensor_scalar_add(t2[:, :w], ps2[:, :w],
                                                    b2_sb[:, m:m + 1])
                        nc.vector.tensor_mul(s_sb[:, :w], s_sb[:, :w], t2[:, :w])
                        nc.sync.dma_start(ht[m * 128:(m + 1) * 128, c0:c0 + w],
                                          s_sb[:, :w])

            # ---- pass 2: ot = gate * (W3^T h + b3), fused epilogue
            with ExitStack() as ctx:
                hp2 = ctx.enter_context(tc.tile_pool(name="hp2", bufs=2))
                w3p = ctx.enter_context(tc.tile_pool(name="w3p", bufs=3))
                op = ctx.enter_context(tc.tile_pool(name="op", bufs=3))
                pps2 = ctx.enter_context(tc.tile_pool(name="pps2", bufs=2, space="PSUM"))
                KF = FF // 128    # 32 k-tiles over FF
                for c0, w in nch:
                    hch = hp2.tile([128, KF * 512], MM_DT, tag="hch")
                    for k2 in range(KF):
                        nc.sync.dma_start(hch[:, k2 * 512:k2 * 512 + w],
                                          _mm(ht[k2 * 128:(k2 + 1) * 128, c0:c0 + w]))
                    for m2 in range(D // 128):
                        w3t = w3p.tile([128, KF * 128], MM_DT, tag="w3t")
                        for k2 in range(KF):
                            nc.sync.dma_start(
                                w3t[:, k2 * 128:(k2 + 1) * 128],
                                _mm(W3.ap()[k2 * 128:(k2 + 1) * 128,
                                            m2 * 128:(m2 + 1) * 128]))
                        ps3 = pps2.tile([128, 512], F32, tag="ps3")
                        for k2 in range(KF):
                            nc.tensor.matmul(
                                ps3[:, :w],
                                w3t[:, k2 * 128:(k2 + 1) * 128],
                                hch[:, k2 * 512:k2 * 512 + w],
                                start=(k2 == 0), stop=(k2 == KF - 1))
                        ot = op.tile([128, 512], F32, tag="ot")
                        nc.vector.tensor_scalar_add(ot[:, :w], ps3[:, :w],
                                                    b3_sb[:, m2:m2 + 1])
                        nc.vector.tensor_mul(ot[:, :w], ot[:, :w], g_bc[:, c0:c0 + w])
                        nc.sync.dma_start(ot_out.ap()[m2 * 128:(m2 + 1) * 128,
                                                      c0:c0 + w], ot[:, :w])

    nc.compile()
    return nc


# ---------------------------------------------------------------- host glue

def _get_attn():
    if "A" not in _CACHE:
        _CACHE["A"] = build_attn()
    return _CACHE["A"]


def _get_moe(C):
    key = ("B", C)
    if key not in _CACHE:
        _CACHE[key] = build_moe(C)
    return _CACHE[key]


def _rope_tables():
    half = D // 2
    theta = (1.0 / (10000.0 ** (np.arange(half, dtype=np.float32) / half)))
    pos = np.arange(S, dtype=np.float32)
    ang = pos[:, None] * theta[None, :]
    return np.cos(ang).astype(np.float32), np.sin(ang).astype(np.float32)


def kernel(x, norm1_w, norm2_w, Wq, Wk, Wv, Wo, router_w, router_b,
           W1, b1, W2, b2, W3, b3, top_k, _trace=False, _prof=None):
    assert int(top_k) == 2, "kernel hardcodes top_k=2"
    f = np.ascontiguousarray
    x = np.asarray(x, dtype=np.float32)
    cos_t, sin_t = _rope_tables()

    # ---- launch A
    nc_a = _get_attn()
    in_maps_a = []
    for c in range(N_CORES):
        b, g = c // 4, c % 4
        r = c % 4
        in_maps_a.append({
            "x_b": f(x[b]),
            "x_mine": f(x[b, r * SQ:(r + 1) * SQ]),
            "Wq_g": f(np.asarray(Wq, np.float32)[:, g * GD:(g + 1) * GD]),
            "Wk_g": f(np.asarray(Wk, np.float32)[:, g * DK:(g + 1) * DK]),
            "Wv_g": f(np.asarray(Wv, np.float32)[:, g * DK:(g + 1) * DK]),
            "Wo_g": f(np.asarray(Wo, np.float32)[g * GD:(g + 1) * GD, :]),
            "n1w": np.asarray(norm1_w, np.float32).reshape(1, D),
            "n2w": np.asarray(norm2_w, np.float32).reshape(1, D),
            "cos_t": cos_t, "sin_t": sin_t,
            "rw": np.asarray(router_w, np.float32),
            "rb": np.asarray(router_b, np.float32).reshape(E, 1),
        })
    res_a = run_bass_kernel_spmd(nc_a, in_maps_a, core_ids=list(range(N_CORES)),
                                 trace=_trace)
    if _prof is not None:
        _prof["A"] = res_a.exec_time_ns

    N = B * S
    h = np.empty((N, D), np.float32)
    y_t = np.empty((D, N), np.float32)
    logits = np.empty((N, E), np.float32)
    for c in range(N_CORES):
        b, r = c // 4, c % 4
        sl = slice(b * S + r * SQ, b * S + (r + 1) * SQ)
        h[sl] = res_a.results[c]["h_out"]
        y_t[:, sl] = res_a.results[c]["yt_out"]
        logits[sl] = res_a.results[c]["lg_out"].T

    if _prof is not None:
        _prof["h"] = h
        _prof["y_t"] = y_t
        _prof["logits"] = logits

    # ---- routing (same math as jax.lax.top_k + softmax over the top-2)
    order = np.argsort(-logits, axis=-1, kind="stable")[:, :2]
    l12 = np.take_along_axis(logits, order, axis=-1)
    ew = np.exp(l12 - l12[:, :1])
    gate_w = ew / ew.sum(-1, keepdims=True)

    idx_list, gate_list = [], []
    for e in range(E):
        sel = (order == e)
        tok = np.nonzero(sel.any(-1))[0]
        gv = gate_w[sel.any(-1)][sel[sel.any(-1)]]  # gate of e for those tokens
        idx_list.append(tok.astype(np.int64))
        gate_list.append(gv.astype(np.float32))
    max_cnt = max(len(ix) for ix in idx_list)
    # multiple of 256 so the pass-2 matmul N-tiling has no partial tiles
    C = max(256, ((max_cnt + 255) // 256) * 256)

    # ---- launch B
    nc_b = _get_moe(C)
    W1 = np.asarray(W1, np.float32)
    W2 = np.asarray(W2, np.float32)
    W3 = np.asarray(W3, np.float32)
    b1 = np.asarray(b1, np.float32)
    b2 = np.asarray(b2, np.float32)
    b3 = np.asarray(b3, np.float32)
    in_maps_b = []
    for e in range(E):
        ix, gv = idx_list[e], gate_list[e]
        Yt_e = np.zeros((D, C), np.float32)
        Yt_e[:, :len(ix)] = y_t[:, ix]
        g_e = np.zeros((1, C), np.float32)
        g_e[0, :len(ix)] = gv
        in_maps_b.append({
            "Yt": Yt_e, "gates": g_e,
            "W1": f(W1[e]), "W2": f(W2[e]), "W3": f(W3[e]),
            "b1": f(b1[e].reshape(FF // 128, 128).T),
            "b2": f(b2[e].reshape(FF // 128, 128).T),
            "b3": f(b3[e].reshape(D // 128, 128).T),
        })
    res_b = run_bass_kernel_spmd(nc_b, in_maps_b, core_ids=list(range(N_CORES)),
                                 trace=_trace)
    if _prof is not None:
        _prof["B"] = res_b.exec_time_ns

    # ---- combine: out = h + sum_e scatter(gated expert outputs)
    out = h.copy()
    for e in range(E):
        ix = idx_list[e]
        if len(ix):
            out[ix] += res_b.results[e]["ot_out"][:, :len(ix)].T
    return out.reshape(B, S, D)


# revision 12
# speedup vs baseline: 1.3521x; 1.3521x over previous
"""Trainium2 Bass kernel for a pre-norm decoder block with GQA attention + top-2 MoE.

Strategy (8 NeuronCores):
  Launch A (attention): shard by (batch, kv-group): core c -> batch c//4, group c%4.
    Each core computes RMSNorm+RoPE for its batch, Q/K/V for its 4 heads /
    1 kv-group over all 2048 tokens, causal attention, and the partial
    output projection attn_g @ Wo[group rows].  Partials are summed with a
    ReduceScatter over the 4 cores of each batch, so core c ends with the
    attention-block output h for its quarter of the sequence, plus the
    RMSNorm'd y (transposed) and router logits for those tokens.
  Host: top-2 routing on the logits (pure index/gather glue), builds the
    per-expert token batches (expert parallelism, "all-to-all" dispatch done
    as the launch-B input sharding).
  Launch B (MoE FFN): core e -> expert e.  Dense SwiGLU FFN over the tokens
    routed to that expert (padded to capacity C), gate-scaled on device.
  Host: scatter-add the gated expert outputs back and add the residual.

Matmuls run with float32 storage; TensorEngine dtype is configurable below
(float32r = full-rate fp32 mode).
"""

import math
import os
from contextlib import ExitStack

import numpy as np

import concourse.bass as bass
import concourse.mybir as mybir
import concourse.tile as tile
from concourse import bacc
from concourse.bass_utils import run_bass_kernel_spmd
from concourse.kernels.tile_matmul import matmul_tile_kernel
from concourse.masks import make_causal_mask, make_identity

AF = mybir.ActivationFunctionType
F32 = mybir.dt.float32
X = mybir.AxisListType.X

# model dims (hardcoded for this problem)
B, S, D = 2, 2048, 1024
NH, G, DK = 16, 4, 64
HPG = NH // G          # 4 heads per group
GD = HPG * DK          # 256 = per-group head width
FF, E = 4096, 8
EPS = 1e-6
N_CORES = 8
SQ = S // 4            # 512 tokens per core after reduce-scatter
NT = S // 128          # 16 token tiles per batch

# TensorEngine compute dtype for the heavy matmuls (storage stays fp32).
MM_DT = mybir.dt.float32r
# dtype for attention score/AV matmuls
ATT_DT = mybir.dt.float32r
BF16 = mybir.dt.bfloat16

_CACHE = {}


def _mm(ap, dt=MM_DT):
    return ap.bitcast(dt) if dt != F32 else ap


# ---------------------------------------------------------------- launch A

def _rmsnorm_tile(nc, pool, src_ap, wbc, tag, eps_ap):
    """token-major rmsnorm of a [128, D] tile; returns normalized sbuf tile."""
    sq = pool.tile([128, D], F32, tag=f"{tag}_sq")
    nc.vector.tensor_mul(sq[:], src_ap, src_ap)
    ssum = pool.tile([128, 1], F32, tag=f"{tag}_ssum")
    nc.vector.reduce_sum(ssum[:], sq[:], axis=X)
    rms = pool.tile([128, 1], F32, tag=f"{tag}_rms")
    nc.scalar.activation(rms[:], ssum[:], AF.Sqrt, bias=eps_ap, scale=1.0 / D)
    rinv = pool.tile([128, 1], F32, tag=f"{tag}_rinv")
    nc.vector.reciprocal(rinv[:], rms[:])
    xn = pool.tile([128, D], F32, tag=f"{tag}_xn")
    nc.scalar.activation(xn[:], src_ap, AF.Copy, scale=rinv[:])
    nc.vector.tensor_mul(xn[:], xn[:], wbc[:])
    return xn


def build_attn():
    nc = bacc.Bacc("TRN2", target_bir_lowering=False, debug=False,
                   num_devices=N_CORES)
    x_b = nc.declare_dram_parameter("x_b", [S, D], F32, False)
    x_mine = nc.declare_dram_parameter("x_mine", [SQ, D], F32, False)
    Wq_g = nc.declare_dram_parameter("Wq_g", [D, GD], F32, False)
    Wk_g = nc.declare_dram_parameter("Wk_g", [D, DK], F32, False)
    Wv_g = nc.declare_dram_parameter("Wv_g", [D, DK], F32, False)
    Wo_g = nc.declare_dram_parameter("Wo_g", [GD, D], F32, False)
    n1w = nc.declare_dram_parameter("n1w", [1, D], F32, False)
    n2w = nc.declare_dram_parameter("n2w", [1, D], F32, False)
    cos_t = nc.declare_dram_parameter("cos_t", [S, D // 2], F32, False)
    sin_t = nc.declare_dram_parameter("sin_t", [S, D // 2], F32, False)
    rw = nc.declare_dram_parameter("rw", [D, E], F32, False)
    rb = nc.declare_dram_parameter("rb", [E, 1], F32, False)
    h_out = nc.declare_dram_parameter("h_out", [SQ, D], F32, True)
    yt_out = nc.declare_dram_parameter("yt_out", [D, SQ], F32, True)
    lg_out = nc.declare_dram_parameter("lg_out", [E, SQ], F32, True)

    with tile.TileContext(nc) as tc:
        with ExitStack() as top:
            dram = top.enter_context(tc.tile_pool(name="dram", bufs=1, space="DRAM"))
            xr_t = dram.tile([D, S], F32)
            q_t = dram.tile([GD, S], F32)
            k_t = dram.tile([DK, S], F32)
            v_tm = dram.tile([S, DK], F32)
            attn_t = dram.tile([GD, S], F32)
            partial_o = dram.tile([S, D], F32)
            rs_out = dram.tile([SQ, D], F32)

            cpool = top.enter_context(tc.tile_pool(name="const", bufs=1))
            ident = cpool.tile([128, 128], F32)
            make_identity(nc, ident[:])
            dmask = cpool.tile([128, 128], F32)
            make_causal_mask(nc, dmask[:], mask_val=-1e9)
            w1r = cpool.tile([1, D], F32)
            nc.sync.dma_start(w1r[:], n1w.ap())
            w1bc = cpool.tile([128, D], F32)
            nc.gpsimd.partition_broadcast(w1bc[:], w1r[:1, :])
            w2r = cpool.tile([1, D], F32)
            nc.sync.dma_start(w2r[:], n2w.ap())
            w2bc = cpool.tile([128, D], F32)
            nc.gpsimd.partition_broadcast(w2bc[:], w2r[:1, :])
            rw_sb = cpool.tile([128, 8 * E], ATT_DT)
            for kd in range(8):
                nc.sync.dma_start(rw_sb[:, kd * E:(kd + 1) * E],
                                  _mm(rw.ap()[kd * 128:(kd + 1) * 128, :], ATT_DT))
            rb_sb = cpool.tile([E, 1], F32)
            nc.sync.dma_start(rb_sb[:], rb.ap())
            eps_sb = cpool.tile([128, 1], F32)
            nc.gpsimd.memset(eps_sb[:], float(EPS))

            # ---- phase 1: rmsnorm + rope + transpose -> xr_t [D, S]
            with ExitStack() as ctx:
                pool = ctx.enter_context(tc.tile_pool(name="nrm", bufs=3))
                pp = ctx.enter_context(tc.tile_pool(name="nrm_ps", bufs=4, space="PSUM"))
                for it in range(NT):
                    xt = pool.tile([128, D], F32, tag="xt")
                    nc.sync.dma_start(xt[:], x_b.ap()[it * 128:(it + 1) * 128, :])
                    xn = _rmsnorm_tile(nc, pool, xt[:], w1bc, "n1", eps_sb[:])
                    cos = pool.tile([128, D // 2], F32, tag="cos")
                    nc.sync.dma_start(cos[:], cos_t.ap()[it * 128:(it + 1) * 128, :])
                    sin = pool.tile([128, D // 2], F32, tag="sin")
                    nc.sync.dma_start(sin[:], sin_t.ap()[it * 128:(it + 1) * 128, :])
                    xe, xo = xn[:, 0::2], xn[:, 1::2]
                    xr = pool.tile([128, D], F32, tag="xr")
                    t1 = pool.tile([128, D // 2], F32, tag="t1")
                    t2 = pool.tile([128, D // 2], F32, tag="t2")
                    nc.vector.tensor_mul(t1[:], xe, cos[:])
                    nc.vector.tensor_mul(t2[:], xo, sin[:])
                    nc.vector.tensor_sub(xr[:, :D // 2], t1[:], t2[:])
                    nc.vector.tensor_mul(t1[:], xe, sin[:])
                    nc.vector.tensor_mul(t2[:], xo, cos[:])
                    nc.vector.tensor_add(xr[:, D // 2:], t1[:], t2[:])
                    for kd in range(8):
                        pt = pp.tile([128, 128], F32, tag="pt")
                        nc.tensor.transpose(pt[:], xr[:, kd * 128:(kd + 1) * 128], ident[:])
                        st = pool.tile([128, 128], F32, tag="st")
                        nc.scalar.copy(st[:], pt[:])
                        nc.sync.dma_start(
                            xr_t[kd * 128:(kd + 1) * 128, it * 128:(it + 1) * 128], st[:])

            # ---- phase 2: projections
            matmul_tile_kernel(
                tc, _mm(Wq_g.ap()), _mm(xr_t[:]), q_t[:],
                post_mxn_tile_fn=lambda nc_, sb, md, _:
                    nc_.scalar.mul(sb, sb, 1.0 / math.sqrt(DK)))
            matmul_tile_kernel(tc, _mm(Wk_g.ap()), _mm(xr_t[:]), k_t[:])
            matmul_tile_kernel(tc, _mm(xr_t[:]), _mm(Wv_g.ap()), v_tm[:])

            # ---- phase 3: causal attention for 4 heads
            with ExitStack() as ctx:
                apool = ctx.enter_context(tc.tile_pool(name="att_in", bufs=1))
                spool = ctx.enter_context(tc.tile_pool(name="att_wk", bufs=3))
                pp_s = ctx.enter_context(tc.tile_pool(name="ps_s", bufs=2, space="PSUM"))
                pp_t = ctx.enter_context(tc.tile_pool(name="ps_t", bufs=2, space="PSUM"))
                pp_av = ctx.enter_context(tc.tile_pool(name="ps_av", bufs=2, space="PSUM"))

                q_sb = []
                for h in range(HPG):
                    qh = apool.tile([DK, S], ATT_DT, name=f"q_sb{h}")
                    nc.sync.dma_start(qh[:], _mm(q_t[h * DK:(h + 1) * DK, :], ATT_DT))
                    q_sb.append(qh)
                k_sb = apool.tile([DK, S], ATT_DT)
                nc.sync.dma_start(k_sb[:], _mm(k_t[:], ATT_DT))
                v_sb = apool.tile([128, NT * DK], ATT_DT)
                for kt in range(NT):
                    nc.sync.dma_start(v_sb[:, kt * DK:(kt + 1) * DK],
                                      _mm(v_tm[kt * 128:(kt + 1) * 128, :], ATT_DT))

                for j in range(NT):
                    S_k = (j + 1) * 128
                    attn_sb = spool.tile([128, 2 * 128], F32, tag="attn_sb")
                    for h in range(HPG):
                        sc = spool.tile([128, S], F32, tag="sc")
                        for c0 in range(0, S_k, 512):
                            w = min(512, S_k - c0)
                            ps = pp_s.tile([128, 512], F32, tag="ps")
                            nc.tensor.matmul(
                                ps[:, :w],
                                q_sb[h][:, j * 128:(j + 1) * 128],
                                k_sb[:, c0:c0 + w],
                                start=True, stop=True)
                            if c0 + w == S_k:
                                if w > 128:
                                    nc.scalar.copy(sc[:, c0:c0 + w - 128], ps[:, :w - 128])
                                nc.vector.tensor_add(sc[:, S_k - 128:S_k],
                                                     ps[:, w - 128:w], dmask[:])
                            else:
                                nc.scalar.copy(sc[:, c0:c0 + w], ps[:, :w])
                        negm = spool.tile([128, 1], F32, tag="negm")
                        nc.vector.reduce_max(negm[:], sc[:, :S_k], axis=X, negate=True)
                        nc.scalar.activation(sc[:, :S_k], sc[:, :S_k], AF.Exp, bias=negm[:])
                        ssum = spool.tile([128, 1], F32, tag="ssum2")
                        nc.vector.reduce_sum(ssum[:], sc[:, :S_k], axis=X)
                        rinv = spool.tile([128, 1], F32, tag="rinv2")
                        nc.vector.reciprocal(rinv[:], ssum[:])
                        nc.scalar.activation(sc[:, :S_k], sc[:, :S_k], AF.Copy, scale=rinv[:])
                        ptb = spool.tile([128, S], ATT_DT, tag="ptb")
                        for kt in range(j + 1):
                            pst = pp_t.tile([128, 128], F32, tag="pst")
                            nc.tensor.transpose(pst[:], sc[:, kt * 128:(kt + 1) * 128],
                                                ident[:])
                            nc.scalar.copy(ptb[:, kt * 128:(kt + 1) * 128], pst[:])
                        av = pp_av.tile([DK, 128], F32, tag="av")
                        for kt in range(j + 1):
                            nc.tensor.matmul(
                                av[:],
                                v_sb[:, kt * DK:(kt + 1) * DK],
                                ptb[:, kt * 128:(kt + 1) * 128],
                                start=(kt == 0), stop=(kt == j))
                        nc.scalar.copy(
                            attn_sb[(h % 2) * DK:(h % 2 + 1) * DK,
                                    (h // 2) * 128:(h // 2 + 1) * 128], av[:])
                    nc.sync.dma_start(attn_t[0:128, j * 128:(j + 1) * 128],
                                      attn_sb[:, 0:128])
                    nc.sync.dma_start(attn_t[128:256, j * 128:(j + 1) * 128],
                                      attn_sb[:, 128:256])

            # ---- phase 4: partial output projection
            matmul_tile_kernel(tc, _mm(attn_t[:]), _mm(Wo_g.ap()), partial_o[:])

            # ---- phase 5: reduce-scatter over the 4 cores of each batch
            nc.gpsimd.collective_compute(
                "ReduceScatter", mybir.AluOpType.add,
                replica_groups=[[0, 1, 2, 3], [4, 5, 6, 7]],
                ins=[partial_o[:].opt()], outs=[rs_out[:].opt()])

            # ---- phase 6: h = x + attn; y = rmsnorm(h); y_t; logits
            with ExitStack() as ctx:
                pool = ctx.enter_context(tc.tile_pool(name="tail", bufs=3))
                pp = ctx.enter_context(tc.tile_pool(name="tail_ps", bufs=4, space="PSUM"))
                pp_l = ctx.enter_context(tc.tile_pool(name="tail_pl", bufs=2, space="PSUM"))
                for it in range(SQ // 128):
                    ht = pool.tile([128, D], F32, tag="ht")
                    nc.sync.dma_start(ht[:], rs_out[it * 128:(it + 1) * 128, :])
                    xm = pool.tile([128, D], F32, tag="xm")
                    nc.sync.dma_start(xm[:], x_mine.ap()[it * 128:(it + 1) * 128, :])
                    nc.vector.tensor_add(ht[:], ht[:], xm[:])
                    nc.sync.dma_start(h_out.ap()[it * 128:(it + 1) * 128, :], ht[:])
                    yt = _rmsnorm_tile(nc, pool, ht[:], w2bc, "n2", eps_sb[:])
                    st = pool.tile([128, 8 * 128], ATT_DT, tag="st2")
                    for kd in range(8):
                        pt = pp.tile([128, 128], F32, tag="pt2")
                        nc.tensor.transpose(pt[:], yt[:, kd * 128:(kd + 1) * 128], ident[:])
                        nc.scalar.copy(st[:, kd * 128:(kd + 1) * 128], pt[:])
                        nc.sync.dma_start(
                            _mm(yt_out.ap()[kd * 128:(kd + 1) * 128,
                                            it * 128:(it + 1) * 128], ATT_DT),
                            st[:, kd * 128:(kd + 1) * 128])
                    psl = pp_l.tile([E, 128], F32, tag="pl")
                    for kd in range(8):
                        nc.tensor.matmul(
                            psl[:],
                            rw_sb[:, kd * E:(kd + 1) * E],
                            st[:, kd * 128:(kd + 1) * 128],
                            start=(kd == 0), stop=(kd == 7))
                    lg = pool.tile([E, 128], F32, tag="lg")
                    nc.vector.tensor_scalar_add(lg[:], psl[:], rb_sb[:, 0:1])
                    nc.sync.dma_start(lg_out.ap()[:, it * 128:(it + 1) * 128], lg[:])

    nc.compile()
    return nc


# ---------------------------------------------------------------- launch B

def build_moe(C):
    """Per-core expert FFN: bf16 matmuls, fp32 accumulation and epilogues.

    Weights arrive pre-cast to bf16 host-side; Yt (gathered tokens) too.
    """
    nc = bacc.Bacc("TRN2", target_bir_lowering=False, debug=False,
                   num_devices=N_CORES)
    Yt = nc.declare_dram_parameter("Yt", [D, C], BF16, False)
    gates = nc.declare_dram_parameter("gates", [1, C], F32, False)
    W1 = nc.declare_dram_parameter("W1", [D, FF], BF16, False)
    W2 = nc.declare_dram_parameter("W2", [D, FF], BF16, False)
    W3 = nc.declare_dram_parameter("W3", [FF, D], BF16, False)
    b1 = nc.declare_dram_parameter("b1", [128, FF // 128], F32, False)
    b2 = nc.declare_dram_parameter("b2", [128, FF // 128], F32, False)
    b3 = nc.declare_dram_parameter("b3", [128, D // 128], F32, False)
    ot_out = nc.declare_dram_parameter("ot_out", [D, C], F32, True)

    KD = D // 128     # 8 k-tiles over D
    KF = FF // 128    # 32 k-tiles over FF
    nch = [(c0, min(512, C - c0)) for c0 in range(0, C, 512)]

    with tile.TileContext(nc) as tc:
        with ExitStack() as top:
            dram = top.enter_context(tc.tile_pool(name="dram", bufs=1, space="DRAM"))
            ht = dram.tile([FF, C], BF16)

            cpool = top.enter_context(tc.tile_pool(name="const", bufs=1))
            b1_sb = cpool.tile([128, MF := FF // 128], F32)
            nc.sync.dma_start(b1_sb[:], b1.ap())
            b2_sb = cpool.tile([128, MF], F32)
            nc.sync.dma_start(b2_sb[:], b2.ap())
            b3_sb = cpool.tile([128, D // 128], F32)
            nc.sync.dma_start(b3_sb[:], b3.ap())
            g_row = cpool.tile([1, C], F32)
            nc.sync.dma_start(g_row[:], gates.ap())
            g_bc = cpool.tile([128, C], F32)
            nc.gpsimd.partition_broadcast(g_bc[:], g_row[:1, :])

            # ---- pass 1: ht = silu(W1^T y + b1) * (W2^T y + b2)   [FF, C] bf16
            with ExitStack() as ctx:
                ypool = ctx.enter_context(tc.tile_pool(name="yp", bufs=1))
                yt_sb = ypool.tile([128, KD * C], BF16)
                for k in range(KD):
                    nc.sync.dma_start(yt_sb[:, k * C:(k + 1) * C],
                                      Yt.ap()[k * 128:(k + 1) * 128, :])
                wpool = ctx.enter_context(tc.tile_pool(name="wp", bufs=3))
                hpool = ctx.enter_context(tc.tile_pool(name="hp", bufs=3))
                pps = ctx.enter_context(tc.tile_pool(name="pps", bufs=4, space="PSUM"))
                # M-chunks of 512 over FF; [128, 4, 512] weight tiles keep the
                # DMA descriptors at 512 contiguous elements per partition.
                for mc in range(FF // 512):
                    wts = []
                    for wsrc in (W1, W2):
                        for kc in range(D // 512):
                            wt = wpool.tile([128, 4, 512], BF16, tag=f"w{len(wts)}")
                            nc.sync.dma_start(
                                wt[:],
                                wsrc.ap()[kc * 512:(kc + 1) * 512,
                                          mc * 512:(mc + 1) * 512]
                                .rearrange("(ko ki) m -> ki ko m", ki=128))
                            wts.append(wt)
                    w1ts, w2ts = wts[:2], wts[2:]
                    for ms in range(4):
                        m = mc * 4 + ms
                        for c0, w in nch:
                            ps1 = pps.tile([128, 512], F32, tag="ps1")
                            for k in range(KD):
                                nc.tensor.matmul(
                                    ps1[:, :w],
                                    w1ts[k // 4][:, k % 4, ms * 128:(ms + 1) * 128],
                                    yt_sb[:, k * C + c0:k * C + c0 + w],
                                    start=(k == 0), stop=(k == KD - 1))
                            ps2 = pps.tile([128, 512], F32, tag="ps2")
                            for k in range(KD):
                                nc.tensor.matmul(
                                    ps2[:, :w],
                                    w2ts[k // 4][:, k % 4, ms * 128:(ms + 1) * 128],
                                    yt_sb[:, k * C + c0:k * C + c0 + w],
                                    start=(k == 0), stop=(k == KD - 1))
                            s_sb = hpool.tile([128, 512], F32, tag="s_sb")
                            nc.scalar.activation(s_sb[:, :w], ps1[:, :w], AF.Silu,
                                                 bias=b1_sb[:, m:m + 1])
                            t2 = hpool.tile([128, 512], F32, tag="t2")
                            nc.vector.tensor_scalar_add(t2[:, :w], ps2[:, :w],
                                                        b2_sb[:, m:m + 1])
                            hb = hpool.tile([128, 512], BF16, tag="hb")
                            nc.vector.tensor_mul(hb[:, :w], s_sb[:, :w], t2[:, :w])
                            nc.sync.dma_start(ht[m * 128:(m + 1) * 128, c0:c0 + w],
                                              hb[:, :w])

            # ---- pass 2: ot = gate * (W3^T h + b3); W3 streamed as k-slabs,
            # 8 concurrent psum accumulation groups (one per 128-wide out tile)
            with ExitStack() as ctx:
                hp2 = ctx.enter_context(tc.tile_pool(name="hp2", bufs=2))
                w3p = ctx.enter_context(tc.tile_pool(name="w3p", bufs=3))
                op = ctx.enter_context(tc.tile_pool(name="op", bufs=3))
                pps2 = ctx.enter_context(tc.tile_pool(name="pps2", bufs=1, space="PSUM"))
                for c0, w in nch:
                    hch = hp2.tile([128, KF * 512], BF16, tag="hch")
                    for k2 in range(KF):
                        nc.sync.dma_start(hch[:, k2 * 512:k2 * 512 + w],
                                          ht[k2 * 128:(k2 + 1) * 128, c0:c0 + w])
                    ps3 = [pps2.tile([128, 512], F32, tag=f"ps3_{m2}", name=f"ps3_{m2}")
                           for m2 in range(D // 128)]
                    for k2 in range(KF):
                        w3s = w3p.tile([128, D], BF16, tag="w3s")
                        nc.sync.dma_start(w3s[:], W3.ap()[k2 * 128:(k2 + 1) * 128, :])
                        for m2 in range(D // 128):
                            nc.tensor.matmul(
                                ps3[m2][:, :w],
                                w3s[:, m2 * 128:(m2 + 1) * 128],
                                hch[:, k2 * 512:k2 * 512 + w],
                                start=(k2 == 0), stop=(k2 == KF - 1))
                    for m2 in range(D // 128):
                        ot = op.tile([128, 512], F32, tag="ot")
                        nc.vector.tensor_scalar_add(ot[:, :w], ps3[m2][:, :w],
                                                    b3_sb[:, m2:m2 + 1])
                        nc.vector.tensor_mul(ot[:, :w], ot[:, :w], g_bc[:, c0:c0 + w])
                        nc.sync.dma_start(ot_out.ap()[m2 * 128:(m2 + 1) * 128,
                                                      c0:c0 + w], ot[:, :w])

    nc.compile()
    return nc


# ---------------------------------------------------------------- host glue

def _get_attn():
    if "A" not in _CACHE:
        _CACHE["A"] = build_attn()
    return _CACHE["A"]


def _get_moe(C):
    key = ("B", C)
    if key not in _CACHE:
        _CACHE[key] = build_moe(C)
    return _CACHE[key]


def _rope_tables():
    half = D // 2
    theta = (1.0 / (10000.0 ** (np.arange(half, dtype=np.float32) / half)))
    pos = np.arange(S, dtype=np.float32)
    ang = pos[:, None] * theta[None, :]
    return np.cos(ang).astype(np.float32), np.sin(ang).astype(np.float32)


def kernel(x, norm1_w, norm2_w, Wq, Wk, Wv, Wo, router_w, router_b,
           W1, b1, W2, b2, W3, b3, top_k, _trace=False, _prof=None):
    assert int(top_k) == 2, "kernel hardcodes top_k=2"
    f = np.ascontiguousarray
    x = np.asarray(x, dtype=np.float32)
    cos_t, sin_t = _rope_tables()

    # ---- launch A
    nc_a = _get_attn()
    in_maps_a = []
    for c in range(N_CORES):
        b, g = c // 4, c % 4
        r = c % 4
        in_maps_a.append({
            "x_b": f(x[b]),
            "x_mine": f(x[b, r * SQ:(r + 1) * SQ]),
            "Wq_g": f(np.asarray(Wq, np.float32)[:, g * GD:(g + 1) * GD]),
            "Wk_g": f(np.asarray(Wk, np.float32)[:, g * DK:(g + 1) * DK]),
            "Wv_g": f(np.asarray(Wv, np.float32)[:, g * DK:(g + 1) * DK]),
            "Wo_g": f(np.asarray(Wo, np.float32)[g * GD:(g + 1) * GD, :]),
            "n1w": np.asarray(norm1_w, np.float32).reshape(1, D),
            "n2w": np.asarray(norm2_w, np.float32).reshape(1, D),
            "cos_t": cos_t, "sin_t": sin_t,
            "rw": np.asarray(router_w, np.float32),
            "rb": np.asarray(router_b, np.float32).reshape(E, 1),
        })
    res_a = run_bass_kernel_spmd(nc_a, in_maps_a, core_ids=list(range(N_CORES)),
                                 trace=_trace)
    if _prof is not None:
        _prof["A"] = res_a.exec_time_ns

    N = B * S
    h = np.empty((N, D), np.float32)
    y_t = np.empty((D, N), np.float32)
    logits = np.empty((N, E), np.float32)
    for c in range(N_CORES):
        b, r = c // 4, c % 4
        sl = slice(b * S + r * SQ, b * S + (r + 1) * SQ)
        h[sl] = res_a.results[c]["h_out"]
        y_t[:, sl] = res_a.results[c]["yt_out"]
        logits[sl] = res_a.results[c]["lg_out"].T

    if _prof is not None:
        _prof["h"] = h
        _prof["y_t"] = y_t
        _prof["logits"] = logits

    # ---- routing (same math as jax.lax.top_k + softmax over the top-2)
    order = np.argsort(-logits, axis=-1, kind="stable")[:, :2]
    l12 = np.take_along_axis(logits, order, axis=-1)
    ew = np.exp(l12 - l12[:, :1])
    gate_w = ew / ew.sum(-1, keepdims=True)

    idx_list, gate_list = [], []
    for e in range(E):
        sel = (order == e)
        tok = np.nonzero(sel.any(-1))[0]
        gv = gate_w[sel.any(-1)][sel[sel.any(-1)]]  # gate of e for those tokens
        idx_list.append(tok.astype(np.int64))
        gate_list.append(gv.astype(np.float32))
    max_cnt = max(len(ix) for ix in idx_list)
    # multiple of 256 so the pass-2 matmul N-tiling has no partial tiles
    C = max(256, ((max_cnt + 255) // 256) * 256)

    # ---- launch B
    import ml_dtypes
    bf = ml_dtypes.bfloat16
    nc_b = _get_moe(C)
    W1 = np.asarray(W1, np.float32)
    W2 = np.asarray(W2, np.float32)
    W3 = np.asarray(W3, np.float32)
    b1 = np.asarray(b1, np.float32)
    b2 = np.asarray(b2, np.float32)
    b3 = np.asarray(b3, np.float32)
    in_maps_b = []
    for e in range(E):
        ix, gv = idx_list[e], gate_list[e]
        Yt_e = np.zeros((D, C), bf)
        Yt_e[:, :len(ix)] = y_t[:, ix].astype(bf)
        g_e = np.zeros((1, C), np.float32)
        g_e[0, :len(ix)] = gv
        in_maps_b.append({
            "Yt": Yt_e, "gates": g_e,
            "W1": f(W1[e].astype(bf)), "W2": f(W2[e].astype(bf)),
            "W3": f(W3[e].astype(bf)),
            "b1": f(b1[e].reshape(FF // 128, 128).T),
            "b2": f(b2[e].reshape(FF // 128, 128).T),
            "b3": f(b3[e].reshape(D // 128, 128).T),
        })
    res_b = run_bass_kernel_spmd(nc_b, in_maps_b, core_ids=list(range(N_CORES)),
                                 trace=_trace)
    if _prof is not None:
        _prof["B"] = res_b.exec_time_ns

    # ---- combine: out = h + sum_e scatter(gated expert outputs)
    out = h.copy()
    for e in range(E):
        ix = idx_list[e]
        if len(ix):
            out[ix] += res_b.results[e]["ot_out"][:, :len(ix)].T
    return out.reshape(B, S, D)
